# revision 23
# baseline (speedup 1.0000x reference)
"""Multi-head attention (N=2, L=2048, D=1024, H=16) on 8 NeuronCores.

Sharding: core c -> (batch n = c // 4, head group g = c % 4, 4 heads each).
Each core computes Q/K/V projections for its 4 heads, flash-style attention
(S^T = K @ Q^T per k-tile, exp on ScalarE with 1/sqrt(D) folded into the
activation scale, P^T @ V via TensorE with a ones-column appended to V to get
the softmax denominator for free), normalization, and its slice of the output
projection. Host sums the 4 partial output projections per batch and adds bo.

All matmul operands are fp16 (full-speed PE, fp32 PSUM accumulate).
"""
import os
import sys
import types

import numpy as np

N_BATCH = 2
L = 2048
D = 1024
H = 16
HD = 64
CORES = 8
GH = 4            # heads per core
DG = GH * HD      # 256 = projected dims per core
QB = 512          # q block
KT = L // 128     # 16 k tiles
QT = L // QB      # 4 q blocks
DC = D // 128     # 8 din chunks
SCALE = 1.0 / 32.0  # 1/sqrt(D)


def _install_ntff_hook():
    """The image's antenv stub lacks axon_hooks; shim it so trace=True works."""
    if "antenv.axon_hooks" in sys.modules:
        return
    mod = types.ModuleType("antenv.axon_hooks")
    mod._hook = None
    mod.set_axon_ntff_profile_hook = lambda h: setattr(mod, "_hook", h)
    mod.get_axon_ntff_profile_hook = lambda: mod._hook
    sys.modules["antenv.axon_hooks"] = mod
    try:
        from trn_agent_boot.trn_boot import _ntff_profile_via_ctypes
        mod._hook = _ntff_profile_via_ctypes("/opt/axon/libaxon_pjrt.so")
    except Exception:
        mod._hook = None


_install_ntff_hook()

import concourse.bacc as bacc
import concourse.mybir as mybir
import concourse.tile as tile
from concourse.bass_utils import run_bass_kernel_spmd

F32 = mybir.dt.float32
F16 = mybir.dt.float16
F32R = mybir.dt.float32r
AF = mybir.ActivationFunctionType
MULT = mybir.AluOpType.mult

_CACHE = {}


_TABLES_PATCHED = False


def _patch_act_tables():
    """Prefer natural_log_exp_and_others so Exp and Ln share one table set."""
    global _TABLES_PATCHED
    if _TABLES_PATCHED:
        return
    import concourse.bacc as _bacc
    import concourse.hw_specs as _hw
    orig_fn = _hw.get_activation_tables

    def patched(arch):
        import concourse.mybir as _mybir
        tabs = dict(orig_fn(arch))
        pref = "natural_log_exp_and_others"
        if pref not in tabs:
            return tabs
        drop = {_mybir.ActivationFunctionType.Exp,
                _mybir.ActivationFunctionType.Ln}
        return {k: (v if k == pref else (set(v) - drop))
                for k, v in tabs.items()}

    _bacc.get_activation_tables = patched
    _TABLES_PATCHED = True


def _build(use_bias, use_mask):
    key = (use_bias, use_mask)
    if key in _CACHE:
        return _CACHE[key]
    if os.environ.get("ACT_TABLE_PATCH", "1") == "1":
        _patch_act_tables()

    nc = bacc.Bacc("TRN2", debug=False, num_devices=CORES)

    xqT = nc.dram_tensor("xqT", [D, L], F16, kind="ExternalInput").ap()
    xkT = nc.dram_tensor("xkT", [D, L], F16, kind="ExternalInput").ap()
    xvT = nc.dram_tensor("xvT", [D, L], F16, kind="ExternalInput").ap()
    aq = nc.dram_tensor("aq", [128, DC * DG], F16, kind="ExternalInput").ap()
    ak = nc.dram_tensor("ak", [128, DC * DG], F16, kind="ExternalInput").ap()
    av = nc.dram_tensor("av", [128, DC * DG], F16, kind="ExternalInput").ap()
    bo = nc.dram_tensor("bo", [128, GH * D], F16, kind="ExternalInput").ap()
    bq = nc.dram_tensor("bq", [1, DG], F16, kind="ExternalInput").ap()
    bk = nc.dram_tensor("bk", [1, DG], F16, kind="ExternalInput").ap()
    bv = nc.dram_tensor("bv", [1, DG], F16, kind="ExternalInput").ap()
    maskf = nc.dram_tensor("maskf", [128, KT], F32, kind="ExternalInput").ap()
    onesd = nc.dram_tensor("onesd", [128, 512], F16, kind="ExternalInput").ap()
    onesd32 = nc.dram_tensor("onesd32", [128, 64], F32R, kind="ExternalInput").ap()
    outp = nc.dram_tensor("outp", [L, D], F32, kind="ExternalOutput").ap()

    with tile.TileContext(nc) as tc:
        _emit(nc, tc, dict(xqT=xqT, xkT=xkT, xvT=xvT, aq=aq, ak=ak, av=av,
                           bo=bo, bq=bq, bk=bk, bv=bv, maskf=maskf, onesd=onesd, onesd32=onesd32,
                           outp=outp),
              use_bias, use_mask)
    nc.compile()
    _CACHE[key] = nc
    return nc


def _emit(nc, tc, t, use_bias, use_mask):
    from contextlib import ExitStack
    ctx = ExitStack()
    with ctx:
        sb_w = ctx.enter_context(tc.tile_pool(name="sb_w", bufs=1))
        sb_qkv = ctx.enter_context(tc.tile_pool(name="sb_qkv", bufs=1))
        sb_pt = ctx.enter_context(tc.tile_pool(name="sb_pt", bufs=4))
        sb_n = ctx.enter_context(tc.tile_pool(name="sb_n", bufs=5))
        sb_out = ctx.enter_context(tc.tile_pool(name="sb_out", bufs=3))
        ps = ctx.enter_context(tc.tile_pool(name="ps", bufs=8, space="PSUM"))

        # ---- resident tiles ----
        ak_t = sb_w.tile([128, DC, DG], F16, tag="ak")
        aq_t = sb_w.tile([128, DC, DG], F16, tag="aq")
        av_t = sb_w.tile([128, DC, DG], F16, tag="av")
        bo_t = sb_w.tile([128, GH, D], F16, tag="bo")
        ones_t = sb_w.tile([128, 512], F16, tag="ones")
        xk_res = sb_w.tile([128, DC, L], F16, tag="xk")
        xq_res = sb_w.tile([128, DC, L], F16, tag="xq")
        xv_res = sb_w.tile([128, DC, L], F16, tag="xv")
        KT_sb = [sb_qkv.tile([128, L], F16, tag=f"kt{m}", name=f"KTm{m}")
                 for m in range(2)]
        QT_sb = [sb_qkv.tile([128, L], F16, tag=f"qt{h}", name=f"QTh{h}")
                 for h in range(GH)]
        V1 = sb_qkv.tile([128, KT, GH, HD + 1], F16, tag="v1")
        oN_sb = [sb_qkv.tile([128, 512], F16, tag=f"oN{h}", name=f"oN{h}")
                 for h in range(GH)]

        # ---- input DMAs: one priority-ordered queue (sync) ----
        # (weights are host-preswizzled to [128, free] partition-contiguous)
        nc.sync.dma_start(out=ak_t, in_=t["ak"].rearrange("p (c d) -> p c d", c=DC))
        for c in range(DC):
            nc.sync.dma_start(out=xk_res[:, c, :],
                              in_=t["xkT"][c * 128:(c + 1) * 128, :])
        nc.sync.dma_start(out=aq_t, in_=t["aq"].rearrange("p (c d) -> p c d", c=DC))
        for c in range(DC):  # qb0 slices of xq first
            nc.sync.dma_start(
                out=xq_res[:, c, 0:512], in_=t["xqT"][c * 128:(c + 1) * 128, 0:512])
        nc.sync.dma_start(out=av_t, in_=t["av"].rearrange("p (c d) -> p c d", c=DC))
        nc.sync.dma_start(out=ones_t, in_=t["onesd"])
        if use_mask:
            mask_t = sb_w.tile([128, KT], F32, tag="mask")
            nc.sync.dma_start(out=mask_t, in_=t["maskf"])
        for c in range(DC):
            nc.sync.dma_start(out=xv_res[:, c, :],
                              in_=t["xvT"][c * 128:(c + 1) * 128, :])
        for qt in range(1, QT):
            for c in range(DC):
                nc.sync.dma_start(
                    out=xq_res[:, c, qt * 512:(qt + 1) * 512],
                    in_=t["xqT"][c * 128:(c + 1) * 128, qt * 512:(qt + 1) * 512])
        nc.sync.dma_start(out=bo_t, in_=t["bo"].rearrange("p (h d) -> p h d", h=GH))
        bq_t = bk_t = bv_t = None
        if use_bias:
            bq_t = sb_w.tile([1, DG], F16, tag="bq")
            bk_t = sb_w.tile([1, DG], F16, tag="bk")
            bv_t = sb_w.tile([1, DG], F16, tag="bv")
            nc.sync.dma_start(out=bq_t, in_=t["bq"])
            nc.sync.dma_start(out=bk_t, in_=t["bk"])
            nc.sync.dma_start(out=bv_t, in_=t["bv"])

        # zero halves of per-head Q^T; oN padding rows
        for h in range(GH):
            z0 = 0 if h % 2 else 64
            nc.vector.memset(QT_sb[h][z0:z0 + 64, :], 0.0)
        for h in range(GH):
            nc.vector.memset(oN_sb[h][64:128, :], 0.0)

        # ACT table warmup (natural_log_exp set covers Exp + Ln)
        warm = sb_w.tile([1, 32], F32, tag="warm")
        nc.vector.memset(warm, 1.0)
        warm2 = sb_w.tile([1, 32], F32, tag="warm2")
        nc.scalar.activation(out=warm2, in_=warm, func=AF.Ln)
        nc.scalar.activation(out=warm2, in_=warm, func=AF.Exp)

        # V1 ones column
        if use_mask:
            ones4 = sb_w.tile([128, GH], F32, tag="ones4")
            nc.vector.memset(ones4, 1.0)
            for kt in range(KT):
                nc.vector.tensor_scalar_mul(
                    V1[:, kt, :, HD:HD + 1],
                    ones4.rearrange("p h -> p h 1"), mask_t[:, kt:kt + 1])
        else:
            nc.sync.dma_start(
                out=V1[:, :, :, HD:HD + 1],
                in_=t["onesd"][:, 0:KT * GH].rearrange(
                    "p (a b c) -> p a b c", a=KT, c=1))

        # ---- emit helpers (all transient PSUM on tag "s", pso on "o") ----
        def emit_kproj(qt):
            psm = [ps.tile([128, 1024], F32, tag="s", bufs=2,
                           name=f"psk_{qt}_{_}") for _ in range(2)]
            for c in range(DC):
                xsl = xk_res[:, c, qt * 512:(qt + 1) * 512]
                for m in range(2):
                    nc.tensor.matmul(
                        psm[m][:, 0:512], ak_t[:, c, m * 128:(m + 1) * 128], xsl,
                        start=(c == 0), stop=(c == DC - 1 and not use_bias))
            if use_bias:
                for m in range(2):
                    nc.tensor.matmul(
                        psm[m][:, 0:512], bk_t[:, m * 128:(m + 1) * 128],
                        ones_t[0:1, :], start=False, stop=True)
            for m in range(2):
                nc.vector.tensor_copy(
                    KT_sb[m][:, qt * 512:(qt + 1) * 512], psm[m][:, 0:512])

        def emit_qproj(qt, hp):
            hs = (hp * 2, hp * 2 + 1)
            psq = {h: ps.tile([128, 1024], F32, tag="s", bufs=2,
                              name=f"psq_{qt}_{h}") for h in hs}
            for c in range(DC):
                xsl = xq_res[:, c, qt * 512:(qt + 1) * 512]
                for h in hs:
                    r0 = 64 * (h % 2)
                    nc.tensor.matmul(
                        psq[h][r0:r0 + 64, 0:512],
                        aq_t[:, c, h * 64:(h + 1) * 64], xsl,
                        start=(c == 0), stop=(c == DC - 1 and not use_bias),
                        tile_position=(0, r0))
            if use_bias:
                for h in hs:
                    r0 = 64 * (h % 2)
                    nc.tensor.matmul(
                        psq[h][r0:r0 + 64, 0:512],
                        bq_t[:, h * 64:(h + 1) * 64], ones_t[0:1, :],
                        start=False, stop=True, tile_position=(0, r0))
            for h in hs:
                r0 = 64 * (h % 2)
                nc.vector.tensor_copy(
                    QT_sb[h][r0:r0 + 64, qt * 512:(qt + 1) * 512],
                    psq[h][r0:r0 + 64, 0:512])

        def emit_vproj(ktg, jp):
            js = (jp * 2, jp * 2 + 1)
            psv = {j: ps.tile([128, 1024], F32, tag="s", bufs=2,
                              name=f"psv_{ktg}_{j}") for j in js}
            for c in range(DC):
                xsl = xv_res[:, c, ktg * 512:(ktg + 1) * 512]
                for j in js:
                    nc.tensor.matmul(
                        psv[j][:, 0:DG], xsl[:, j * 128:(j + 1) * 128],
                        av_t[:, c, :],
                        start=(c == 0), stop=(c == DC - 1 and not use_bias))
            if use_bias:
                for j in js:
                    nc.tensor.matmul(
                        psv[j][:, 0:DG], ones_t[0:1, 0:128], bv_t,
                        start=False, stop=True)
            for j in js:
                kt = ktg * 4 + j
                srcv = psv[j][:, 0:DG].rearrange("p (h d) -> p h d", h=GH)
                if use_mask:
                    nc.vector.tensor_scalar_mul(
                        V1[:, kt, :, 0:HD], srcv, mask_t[:, kt:kt + 1])
                else:
                    nc.vector.tensor_copy(V1[:, kt, :, 0:HD], srcv)

        def emit_attn_sk(qb, sk, pso):
            qs0 = qb * QB
            pss = [ps.tile([128, 1024], F32, tag="s", bufs=2,
                           name=f"pss_{qb}_{sk}_{_}") for _ in range(GH)]
            for hp in range(2):
                for dk in range(2):
                    kt = sk * 2 + dk
                    for hh in range(2):
                        h = hp * 2 + hh
                        nc.tensor.matmul(
                            pss[h][:, dk * 512:(dk + 1) * 512],
                            KT_sb[hp][:, kt * 128:(kt + 1) * 128],
                            QT_sb[h][:, qs0:qs0 + QB],
                            start=True, stop=True)
            pts = []
            for h in range(GH):
                pt = sb_pt.tile([128, 1024], F16, tag="pt",
                                name=f"pt_{qb}_{sk}_{h}")
                nc.scalar.activation(out=pt, in_=pss[h], func=AF.Exp,
                                     scale=SCALE)
                pts.append(pt)
            for h in range(GH):
                for dk in range(2):
                    kt = sk * 2 + dk
                    nc.tensor.matmul(
                        pso[h][0:HD + 1, :], V1[:, kt, h, :],
                        pts[h][:, dk * 512:(dk + 1) * 512],
                        start=(kt == 0), stop=(kt == KT - 1))

        pso_all = {}
        oT_all = {}

        def emit_oT(qb):
            oTs = []
            for h in range(GH):
                oT = sb_n.tile([65, 512], F32, tag="oT", name=f"oT_{qb}_{h}")
                nc.vector.tensor_copy(oT, pso_all[qb][h][0:65, :])
                oTs.append(oT)
            oT_all[qb] = oTs

        def emit_tail(qb):
            qs0 = qb * QB
            oNs = []
            for h in range(GH):
                oT = oT_all[qb][h]
                lnr = sb_n.tile([65, 512], F32, tag="lnr", name=f"lnr_{qb}_{h}")
                nc.scalar.activation(out=lnr[64:65, :], in_=oT[64:65, :],
                                     func=AF.Ln)
                rr = sb_n.tile([65, 512], F16, tag="rr", name=f"rr_{qb}_{h}")
                nc.scalar.activation(out=rr[64:65, :], in_=lnr[64:65, :],
                                     func=AF.Exp, scale=-1.0)
                bc = ps.tile([128, 1024], F32, tag="s", bufs=2,
                             name=f"bc_{qb}_{h}")
                nc.tensor.matmul(
                    bc[0:64, 0:512], ones_t[64:65, 0:64], rr[64:65, :],
                    start=True, stop=True, tile_position=(64, 0))
                oN = oN_sb[h]
                nc.vector.tensor_tensor(oN[0:64, :], oT[0:64, :],
                                        bc[0:64, 0:512], op=MULT)
                oNs.append(oN)
            for mq in range(4):
                psout = [ps.tile([128, 512], F32, tag="o", bufs=4,
                                 name=f"psout_{qb}_{mq}_{_}") for _ in range(2)]
                for h in range(GH):
                    for nb in range(2):
                        nc.tensor.matmul(
                            psout[nb][:, 0:512],
                            oNs[h][:, mq * 128:(mq + 1) * 128],
                            bo_t[:, h, nb * 512:(nb + 1) * 512],
                            start=(h == 0), stop=(h == GH - 1))
                ot = sb_out.tile([128, D], F32, tag="ot", name=f"ot_{qb}_{mq}")
                for nb in range(2):
                    nc.vector.tensor_copy(ot[:, nb * 512:(nb + 1) * 512],
                                          psout[nb][:, 0:512])
                q0 = qs0 + mq * 128
                nc.gpsimd.dma_start(out=t["outp"][q0:q0 + 128, :], in_=ot)

        # ---- schedule: interleave projections with qb0's attention ----
        emit_qproj(0, 0)
        emit_qproj(0, 1)
        pso_all[0] = [ps.tile([128, 512], F32, tag="o", bufs=4,
                              name=f"pso_0_{_}") for _ in range(GH)]
        for g in range(4):
            emit_kproj(g)
            emit_vproj(g, 0)
            emit_vproj(g, 1)
            emit_attn_sk(0, 2 * g, pso_all[0])
            emit_attn_sk(0, 2 * g + 1, pso_all[0])
        emit_oT(0)
        for qb in range(1, QT):
            emit_qproj(qb, 0)
            emit_qproj(qb, 1)
            pso_all[qb] = [ps.tile([128, 512], F32, tag="o", bufs=4,
                                   name=f"pso_{qb}_{_}") for _ in range(GH)]
            for sk in range(KT // 2):
                emit_attn_sk(qb, sk, pso_all[qb])
            emit_oT(qb)
            emit_tail(qb - 1)
        emit_tail(QT - 1)


def _swizzle_a(aT):
    """[D, DG] -> [128, DC*DG]: partition p holds chunks c at (c, :)."""
    return np.ascontiguousarray(
        aT.reshape(DC, 128, DG).transpose(1, 0, 2).reshape(128, DC * DG))


def _pad_bo(boT):
    """[256, D] -> [128, GH*D]: head h cols at h*D, rows 64:128 zero."""
    out = np.zeros((128, GH, D), dtype=np.float16)
    out[0:64, :, :] = boT.reshape(GH, 64, D).transpose(1, 0, 2)
    return np.ascontiguousarray(out.reshape(128, GH * D))


def _prep_inputs(values, key, query, mask, Wv, Wk, Wq, Wo, bv, bk, bq):
    """Build the 8 per-core input maps (host-side shard + layout)."""
    xT = {}
    for n in range(N_BATCH):
        xT[("q", n)] = np.ascontiguousarray(query[n].T.astype(np.float16))
        xT[("k", n)] = np.ascontiguousarray(key[n].T.astype(np.float16))
        xT[("v", n)] = np.ascontiguousarray(values[n].T.astype(np.float16))
    in_maps = []
    for c in range(CORES):
        n, g = divmod(c, CORES // N_BATCH)
        rows = slice(g * DG, (g + 1) * DG)
        mrow = np.ascontiguousarray(
            mask[n, 0, 0, :].astype(np.float32).reshape(KT, 128).T)
        in_maps.append({
            "xqT": xT[("q", n)],
            "xkT": xT[("k", n)],
            "xvT": xT[("v", n)],
            "aq": _swizzle_a(Wq[rows, :].T.astype(np.float16)),
            "ak": _swizzle_a(Wk[rows, :].T.astype(np.float16)),
            "av": _swizzle_a(Wv[rows, :].T.astype(np.float16)),
            "bo": _pad_bo(Wo[:, rows].T.astype(np.float16)),
            "bq": np.ascontiguousarray(bq[None, rows].astype(np.float16)),
            "bk": np.ascontiguousarray(bk[None, rows].astype(np.float16)),
            "bv": np.ascontiguousarray(bv[None, rows].astype(np.float16)),
            "maskf": mrow,
            "onesd": _ONES,
            "onesd32": _ONES32,
        })
    return in_maps


_ONES = np.ones((128, 512), dtype=np.float16)
_ONES32 = np.ones((128, 64), dtype=np.float32)
LAST_EXEC_NS = None


def kernel(values, key, query, mask, Wv, bv, Wk, bk, Wq, bq, Wo, bo,
           trace=False):
    global LAST_EXEC_NS
    values = np.asarray(values, dtype=np.float32)
    key = np.asarray(key, dtype=np.float32)
    query = np.asarray(query, dtype=np.float32)
    mask = np.asarray(mask)
    Wq, Wk, Wv, Wo = (np.asarray(Wq, np.float32), np.asarray(Wk, np.float32),
                      np.asarray(Wv, np.float32), np.asarray(Wo, np.float32))
    bq, bk, bv, bo = (np.asarray(bq, np.float32), np.asarray(bk, np.float32),
                      np.asarray(bv, np.float32), np.asarray(bo, np.float32))

    use_bias = bool(np.any(bq) or np.any(bk) or np.any(bv))
    use_mask = not bool(np.all(np.asarray(mask) == 1))

    nc = _build(use_bias, use_mask)
    in_maps = _prep_inputs(values, key, query, mask, Wv, Wk, Wq, Wo,
                           bv, bk, bq)
    res = run_bass_kernel_spmd(nc, in_maps, core_ids=list(range(CORES)),
                               trace=trace)
    LAST_EXEC_NS = res.exec_time_ns

    out = np.zeros((N_BATCH, L, D), dtype=np.float32)
    for c in range(CORES):
        n = c // (CORES // N_BATCH)
        out[n] += res.results[c]["outp"]
    out += bo[None, None, :]
    return out


# revision 24
# speedup vs baseline: 1.1156x; 1.1156x over previous
"""Multi-head attention (N=2, L=2048, D=1024, H=16) on 8 NeuronCores.

Sharding: core c -> (batch n = c // 4, head group g = c % 4, 4 heads each).
Each core computes Q/K/V projections for its 4 heads, flash-style attention
(S^T = K @ Q^T per k-tile, exp on ScalarE with 1/sqrt(D) folded into the
activation scale, P^T @ V via TensorE with a ones-column appended to V to get
the softmax denominator for free), normalization, and its slice of the output
projection. Host sums the 4 partial output projections per batch and adds bo.

All matmul operands are fp16 (full-speed PE, fp32 PSUM accumulate).
"""
import os
import sys
import types

import numpy as np

N_BATCH = 2
L = 2048
D = 1024
H = 16
HD = 64
CORES = 8
GH = 4            # heads per core
DG = GH * HD      # 256 = projected dims per core
QB = 512          # q block
KT = L // 128     # 16 k tiles
QT = L // QB      # 4 q blocks
DC = D // 128     # 8 din chunks
SCALE = 1.0 / 32.0  # 1/sqrt(D)


def _install_ntff_hook():
    """The image's antenv stub lacks axon_hooks; shim it so trace=True works."""
    if "antenv.axon_hooks" in sys.modules:
        return
    mod = types.ModuleType("antenv.axon_hooks")
    mod._hook = None
    mod.set_axon_ntff_profile_hook = lambda h: setattr(mod, "_hook", h)
    mod.get_axon_ntff_profile_hook = lambda: mod._hook
    sys.modules["antenv.axon_hooks"] = mod
    try:
        from trn_agent_boot.trn_boot import _ntff_profile_via_ctypes
        mod._hook = _ntff_profile_via_ctypes("/opt/axon/libaxon_pjrt.so")
    except Exception:
        mod._hook = None


_install_ntff_hook()

import concourse.bacc as bacc
import concourse.mybir as mybir
import concourse.tile as tile
from concourse.bass_utils import run_bass_kernel_spmd

F32 = mybir.dt.float32
F16 = mybir.dt.float16
F32R = mybir.dt.float32r
AF = mybir.ActivationFunctionType
MULT = mybir.AluOpType.mult

_CACHE = {}


_TABLES_PATCHED = False


def _patch_act_tables():
    """Prefer natural_log_exp_and_others so Exp and Ln share one table set."""
    global _TABLES_PATCHED
    if _TABLES_PATCHED:
        return
    import concourse.bacc as _bacc
    import concourse.hw_specs as _hw
    orig_fn = _hw.get_activation_tables

    def patched(arch):
        import concourse.mybir as _mybir
        tabs = dict(orig_fn(arch))
        pref = "natural_log_exp_and_others"
        if pref not in tabs:
            return tabs
        drop = {_mybir.ActivationFunctionType.Exp,
                _mybir.ActivationFunctionType.Ln}
        return {k: (v if k == pref else (set(v) - drop))
                for k, v in tabs.items()}

    _bacc.get_activation_tables = patched
    _TABLES_PATCHED = True


def _build(use_bias, use_mask):
    key = (use_bias, use_mask)
    if key in _CACHE:
        return _CACHE[key]
    if os.environ.get("ACT_TABLE_PATCH", "1") == "1":
        _patch_act_tables()

    nc = bacc.Bacc("TRN2", debug=False, num_devices=CORES)

    xqT = nc.dram_tensor("xqT", [D, L], F16, kind="ExternalInput").ap()
    xkT = nc.dram_tensor("xkT", [D, L], F16, kind="ExternalInput").ap()
    xvT = nc.dram_tensor("xvT", [D, L], F16, kind="ExternalInput").ap()
    aq = nc.dram_tensor("aq", [128, DC * DG], F16, kind="ExternalInput").ap()
    ak = nc.dram_tensor("ak", [128, DC * DG], F16, kind="ExternalInput").ap()
    av = nc.dram_tensor("av", [128, DC * DG], F16, kind="ExternalInput").ap()
    bo = nc.dram_tensor("bo", [128, GH * D], F16, kind="ExternalInput").ap()
    bq = nc.dram_tensor("bq", [1, DG], F16, kind="ExternalInput").ap()
    bk = nc.dram_tensor("bk", [1, DG], F16, kind="ExternalInput").ap()
    bv = nc.dram_tensor("bv", [1, DG], F16, kind="ExternalInput").ap()
    maskf = nc.dram_tensor("maskf", [128, KT], F32, kind="ExternalInput").ap()
    onesd = nc.dram_tensor("onesd", [128, 512], F16, kind="ExternalInput").ap()
    onesd32 = nc.dram_tensor("onesd32", [128, 64], F32R, kind="ExternalInput").ap()
    outp = nc.dram_tensor("outp", [L, D], F32, kind="ExternalOutput").ap()

    with tile.TileContext(nc) as tc:
        _emit(nc, tc, dict(xqT=xqT, xkT=xkT, xvT=xvT, aq=aq, ak=ak, av=av,
                           bo=bo, bq=bq, bk=bk, bv=bv, maskf=maskf, onesd=onesd, onesd32=onesd32,
                           outp=outp),
              use_bias, use_mask)
    nc.compile()
    _CACHE[key] = nc
    return nc


def _emit(nc, tc, t, use_bias, use_mask):
    from contextlib import ExitStack
    ctx = ExitStack()
    with ctx:
        sb_w = ctx.enter_context(tc.tile_pool(name="sb_w", bufs=1))
        sb_qkv = ctx.enter_context(tc.tile_pool(name="sb_qkv", bufs=1))
        sb_pt = ctx.enter_context(tc.tile_pool(name="sb_pt", bufs=6))
        sb_n = ctx.enter_context(tc.tile_pool(name="sb_n", bufs=5))
        sb_out = ctx.enter_context(tc.tile_pool(name="sb_out", bufs=3))
        ps = ctx.enter_context(tc.tile_pool(name="ps", bufs=8, space="PSUM"))

        # ---- resident tiles ----
        ak_t = sb_w.tile([128, DC, DG], F16, tag="ak")
        aq_t = sb_w.tile([128, DC, DG], F16, tag="aq")
        av_t = sb_w.tile([128, DC, DG], F16, tag="av")
        bo_t = sb_w.tile([128, GH, D], F16, tag="bo")
        ones_t = sb_w.tile([128, 512], F16, tag="ones")
        xk_res = sb_w.tile([128, DC, L], F16, tag="xk")
        xq_res = sb_w.tile([128, DC, L], F16, tag="xq")
        xv_res = sb_w.tile([128, DC, L], F16, tag="xv")
        KT_sb = [sb_qkv.tile([128, L], F16, tag=f"kt{m}", name=f"KTm{m}")
                 for m in range(2)]
        QT_sb = [sb_qkv.tile([128, L], F16, tag=f"qt{h}", name=f"QTh{h}")
                 for h in range(GH)]
        V1 = sb_qkv.tile([128, KT, GH, HD + 1], F16, tag="v1")
        oN_sb = [sb_qkv.tile([128, 512], F16, tag=f"oN{h}", name=f"oN{h}")
                 for h in range(GH)]

        # ---- input DMAs: one priority-ordered queue (sync) ----
        # (weights are host-preswizzled to [128, free] partition-contiguous)
        nc.sync.dma_start(out=ak_t, in_=t["ak"].rearrange("p (c d) -> p c d", c=DC))
        for c in range(DC):
            nc.sync.dma_start(out=xk_res[:, c, :],
                              in_=t["xkT"][c * 128:(c + 1) * 128, :])
        nc.sync.dma_start(out=aq_t, in_=t["aq"].rearrange("p (c d) -> p c d", c=DC))
        for c in range(DC):  # qb0 slices of xq first
            nc.sync.dma_start(
                out=xq_res[:, c, 0:512], in_=t["xqT"][c * 128:(c + 1) * 128, 0:512])
        nc.sync.dma_start(out=av_t, in_=t["av"].rearrange("p (c d) -> p c d", c=DC))
        for c in range(DC):
            nc.sync.dma_start(out=xv_res[:, c, :],
                              in_=t["xvT"][c * 128:(c + 1) * 128, :])
        nc.sync.dma_start(out=ones_t, in_=t["onesd"])
        if use_mask:
            mask_t = sb_w.tile([128, KT], F32, tag="mask")
            nc.sync.dma_start(out=mask_t, in_=t["maskf"])
        for qt in range(1, QT):
            for c in range(DC):
                nc.sync.dma_start(
                    out=xq_res[:, c, qt * 512:(qt + 1) * 512],
                    in_=t["xqT"][c * 128:(c + 1) * 128, qt * 512:(qt + 1) * 512])
        nc.sync.dma_start(out=bo_t, in_=t["bo"].rearrange("p (h d) -> p h d", h=GH))
        bq_t = bk_t = bv_t = None
        if use_bias:
            bq_t = sb_w.tile([1, DG], F16, tag="bq")
            bk_t = sb_w.tile([1, DG], F16, tag="bk")
            bv_t = sb_w.tile([1, DG], F16, tag="bv")
            nc.sync.dma_start(out=bq_t, in_=t["bq"])
            nc.sync.dma_start(out=bk_t, in_=t["bk"])
            nc.sync.dma_start(out=bv_t, in_=t["bv"])

        # zero halves of per-head Q^T; oN padding rows
        for h in range(GH):
            z0 = 0 if h % 2 else 64
            nc.vector.memset(QT_sb[h][z0:z0 + 64, :], 0.0)
        for h in range(GH):
            nc.vector.memset(oN_sb[h][64:128, :], 0.0)

        # ACT table warmup (natural_log_exp set covers Exp + Ln)
        warm = sb_w.tile([1, 32], F32, tag="warm")
        nc.vector.memset(warm, 1.0)
        warm2 = sb_w.tile([1, 32], F32, tag="warm2")
        nc.scalar.activation(out=warm2, in_=warm, func=AF.Ln)
        nc.scalar.activation(out=warm2, in_=warm, func=AF.Exp)

        # V1 ones column
        if use_mask:
            ones4 = sb_w.tile([128, GH], F32, tag="ones4")
            nc.vector.memset(ones4, 1.0)
            for kt in range(KT):
                nc.vector.tensor_scalar_mul(
                    V1[:, kt, :, HD:HD + 1],
                    ones4.rearrange("p h -> p h 1"), mask_t[:, kt:kt + 1])
        else:
            nc.sync.dma_start(
                out=V1[:, :, :, HD:HD + 1],
                in_=t["onesd"][:, 0:KT * GH].rearrange(
                    "p (a b c) -> p a b c", a=KT, c=1))

        # ---- emit helpers (all transient PSUM on tag "s", pso on "o") ----
        def emit_kproj(qt):
            psm = [ps.tile([128, 512], F32, tag="o", bufs=4,
                           name=f"psk_{qt}_{_}") for _ in range(2)]
            for c in range(DC):
                xsl = xk_res[:, c, qt * 512:(qt + 1) * 512]
                for m in range(2):
                    nc.tensor.matmul(
                        psm[m][:, 0:512], ak_t[:, c, m * 128:(m + 1) * 128], xsl,
                        start=(c == 0), stop=(c == DC - 1 and not use_bias))
            if use_bias:
                for m in range(2):
                    nc.tensor.matmul(
                        psm[m][:, 0:512], bk_t[:, m * 128:(m + 1) * 128],
                        ones_t[0:1, :], start=False, stop=True)
            for m in range(2):
                nc.vector.tensor_copy(
                    KT_sb[m][:, qt * 512:(qt + 1) * 512], psm[m][:, 0:512])

        def emit_qproj(qt, hp):
            hs = (hp * 2, hp * 2 + 1)
            psq = {h: ps.tile([128, 512], F32, tag="o", bufs=4,
                              name=f"psq_{qt}_{h}") for h in hs}
            for c in range(DC):
                xsl = xq_res[:, c, qt * 512:(qt + 1) * 512]
                for h in hs:
                    r0 = 64 * (h % 2)
                    nc.tensor.matmul(
                        psq[h][r0:r0 + 64, 0:512],
                        aq_t[:, c, h * 64:(h + 1) * 64], xsl,
                        start=(c == 0), stop=(c == DC - 1 and not use_bias),
                        tile_position=(0, r0))
            if use_bias:
                for h in hs:
                    r0 = 64 * (h % 2)
                    nc.tensor.matmul(
                        psq[h][r0:r0 + 64, 0:512],
                        bq_t[:, h * 64:(h + 1) * 64], ones_t[0:1, :],
                        start=False, stop=True, tile_position=(0, r0))
            for h in hs:
                r0 = 64 * (h % 2)
                nc.vector.tensor_copy(
                    QT_sb[h][r0:r0 + 64, qt * 512:(qt + 1) * 512],
                    psq[h][r0:r0 + 64, 0:512])

        def emit_vproj(ktg, jp):
            js = (jp * 2, jp * 2 + 1)
            psv = {j: ps.tile([128, 512], F32, tag="o", bufs=4,
                              name=f"psv_{ktg}_{j}") for j in js}
            for c in range(DC):
                xsl = xv_res[:, c, ktg * 512:(ktg + 1) * 512]
                for j in js:
                    nc.tensor.matmul(
                        psv[j][:, 0:DG], xsl[:, j * 128:(j + 1) * 128],
                        av_t[:, c, :],
                        start=(c == 0), stop=(c == DC - 1 and not use_bias))
            if use_bias:
                for j in js:
                    nc.tensor.matmul(
                        psv[j][:, 0:DG], ones_t[0:1, 0:128], bv_t,
                        start=False, stop=True)
            for j in js:
                kt = ktg * 4 + j
                srcv = psv[j][:, 0:DG].rearrange("p (h d) -> p h d", h=GH)
                if use_mask:
                    nc.vector.tensor_scalar_mul(
                        V1[:, kt, :, 0:HD], srcv, mask_t[:, kt:kt + 1])
                else:
                    nc.vector.tensor_copy(V1[:, kt, :, 0:HD], srcv)

        def emit_attn_sk(qb, sk, pso):
            qs0 = qb * QB
            pss = [ps.tile([128, 1024], F32, tag="s", bufs=2,
                           name=f"pss_{qb}_{sk}_{_}") for _ in range(GH)]
            for hp in range(2):
                for dk in range(2):
                    kt = sk * 2 + dk
                    for hh in range(2):
                        h = hp * 2 + hh
                        nc.tensor.matmul(
                            pss[h][:, dk * 512:(dk + 1) * 512],
                            KT_sb[hp][:, kt * 128:(kt + 1) * 128],
                            QT_sb[h][:, qs0:qs0 + QB],
                            start=True, stop=True)
            pts = []
            for h in range(GH):
                pt = sb_pt.tile([128, 1024], F16, tag="pt",
                                name=f"pt_{qb}_{sk}_{h}")
                nc.scalar.activation(out=pt, in_=pss[h], func=AF.Exp,
                                     scale=SCALE)
                pts.append(pt)
            for h in range(GH):
                for dk in range(2):
                    kt = sk * 2 + dk
                    nc.tensor.matmul(
                        pso[h][0:HD + 1, :], V1[:, kt, h, :],
                        pts[h][:, dk * 512:(dk + 1) * 512],
                        start=(kt == 0), stop=(kt == KT - 1))

        pso_all = {}
        oT_all = {}

        def emit_oT(qb):
            oTs = []
            for h in range(GH):
                oT = sb_n.tile([65, 512], F32, tag="oT", name=f"oT_{qb}_{h}")
                nc.vector.tensor_copy(oT, pso_all[qb][h][0:65, :])
                oTs.append(oT)
            oT_all[qb] = oTs

        def emit_tail(qb):
            qs0 = qb * QB
            oNs = []
            for h in range(GH):
                oT = oT_all[qb][h]
                lnr = sb_n.tile([65, 512], F32, tag="lnr", name=f"lnr_{qb}_{h}")
                nc.scalar.activation(out=lnr[64:65, :], in_=oT[64:65, :],
                                     func=AF.Ln)
                rr = sb_n.tile([65, 512], F16, tag="rr", name=f"rr_{qb}_{h}")
                nc.scalar.activation(out=rr[64:65, :], in_=lnr[64:65, :],
                                     func=AF.Exp, scale=-1.0)
                bc = ps.tile([128, 512], F32, tag="o", bufs=4,
                             name=f"bc_{qb}_{h}")
                nc.tensor.matmul(
                    bc[0:64, :], ones_t[64:65, 0:64], rr[64:65, :],
                    start=True, stop=True, tile_position=(64, 0))
                oN = oN_sb[h]
                nc.vector.tensor_tensor(oN[0:64, :], oT[0:64, :],
                                        bc[0:64, :], op=MULT)
                oNs.append(oN)
            for mq in range(4):
                psout = [ps.tile([128, 512], F32, tag="o", bufs=4,
                                 name=f"psout_{qb}_{mq}_{_}") for _ in range(2)]
                for h in range(GH):
                    for nb in range(2):
                        nc.tensor.matmul(
                            psout[nb][:, 0:512],
                            oNs[h][:, mq * 128:(mq + 1) * 128],
                            bo_t[:, h, nb * 512:(nb + 1) * 512],
                            start=(h == 0), stop=(h == GH - 1))
                ot = sb_out.tile([128, D], F32, tag="ot", name=f"ot_{qb}_{mq}")
                for nb in range(2):
                    nc.vector.tensor_copy(ot[:, nb * 512:(nb + 1) * 512],
                                          psout[nb][:, 0:512])
                q0 = qs0 + mq * 128
                nc.gpsimd.dma_start(out=t["outp"][q0:q0 + 128, :], in_=ot)

        # ---- schedule: projections, then attention (pss decoupled on "s")
        emit_qproj(0, 0)
        emit_qproj(0, 1)
        for g in range(4):
            emit_kproj(g)
        for g in range(4):
            emit_vproj(g, 0)
            emit_vproj(g, 1)
        pso_all[0] = [ps.tile([128, 512], F32, tag="o", bufs=4,
                              name=f"pso_0_{_}") for _ in range(GH)]
        for sk in range(KT // 2):
            emit_attn_sk(0, sk, pso_all[0])
        emit_oT(0)
        for qb in range(1, QT):
            emit_qproj(qb, 0)
            emit_qproj(qb, 1)
            pso_all[qb] = [ps.tile([128, 512], F32, tag="o", bufs=4,
                                   name=f"pso_{qb}_{_}") for _ in range(GH)]
            for sk in range(KT // 2):
                emit_attn_sk(qb, sk, pso_all[qb])
            emit_oT(qb)
            emit_tail(qb - 1)
        emit_tail(QT - 1)


def _swizzle_a(aT):
    """[D, DG] -> [128, DC*DG]: partition p holds chunks c at (c, :)."""
    return np.ascontiguousarray(
        aT.reshape(DC, 128, DG).transpose(1, 0, 2).reshape(128, DC * DG))


def _pad_bo(boT):
    """[256, D] -> [128, GH*D]: head h cols at h*D, rows 64:128 zero."""
    out = np.zeros((128, GH, D), dtype=np.float16)
    out[0:64, :, :] = boT.reshape(GH, 64, D).transpose(1, 0, 2)
    return np.ascontiguousarray(out.reshape(128, GH * D))


def _prep_inputs(values, key, query, mask, Wv, Wk, Wq, Wo, bv, bk, bq):
    """Build the 8 per-core input maps (host-side shard + layout)."""
    xT = {}
    for n in range(N_BATCH):
        xT[("q", n)] = np.ascontiguousarray(query[n].T.astype(np.float16))
        xT[("k", n)] = np.ascontiguousarray(key[n].T.astype(np.float16))
        xT[("v", n)] = np.ascontiguousarray(values[n].T.astype(np.float16))
    in_maps = []
    for c in range(CORES):
        n, g = divmod(c, CORES // N_BATCH)
        rows = slice(g * DG, (g + 1) * DG)
        mrow = np.ascontiguousarray(
            mask[n, 0, 0, :].astype(np.float32).reshape(KT, 128).T)
        in_maps.append({
            "xqT": xT[("q", n)],
            "xkT": xT[("k", n)],
            "xvT": xT[("v", n)],
            "aq": _swizzle_a(Wq[rows, :].T.astype(np.float16)),
            "ak": _swizzle_a(Wk[rows, :].T.astype(np.float16)),
            "av": _swizzle_a(Wv[rows, :].T.astype(np.float16)),
            "bo": _pad_bo(Wo[:, rows].T.astype(np.float16)),
            "bq": np.ascontiguousarray(bq[None, rows].astype(np.float16)),
            "bk": np.ascontiguousarray(bk[None, rows].astype(np.float16)),
            "bv": np.ascontiguousarray(bv[None, rows].astype(np.float16)),
            "maskf": mrow,
            "onesd": _ONES,
            "onesd32": _ONES32,
        })
    return in_maps


_ONES = np.ones((128, 512), dtype=np.float16)
_ONES32 = np.ones((128, 64), dtype=np.float32)
LAST_EXEC_NS = None


def kernel(values, key, query, mask, Wv, bv, Wk, bk, Wq, bq, Wo, bo,
           trace=False):
    global LAST_EXEC_NS
    values = np.asarray(values, dtype=np.float32)
    key = np.asarray(key, dtype=np.float32)
    query = np.asarray(query, dtype=np.float32)
    mask = np.asarray(mask)
    Wq, Wk, Wv, Wo = (np.asarray(Wq, np.float32), np.asarray(Wk, np.float32),
                      np.asarray(Wv, np.float32), np.asarray(Wo, np.float32))
    bq, bk, bv, bo = (np.asarray(bq, np.float32), np.asarray(bk, np.float32),
                      np.asarray(bv, np.float32), np.asarray(bo, np.float32))

    use_bias = bool(np.any(bq) or np.any(bk) or np.any(bv))
    use_mask = not bool(np.all(np.asarray(mask) == 1))

    nc = _build(use_bias, use_mask)
    in_maps = _prep_inputs(values, key, query, mask, Wv, Wk, Wq, Wo,
                           bv, bk, bq)
    res = run_bass_kernel_spmd(nc, in_maps, core_ids=list(range(CORES)),
                               trace=trace)
    LAST_EXEC_NS = res.exec_time_ns

    out = np.zeros((N_BATCH, L, D), dtype=np.float32)
    for c in range(CORES):
        n = c // (CORES // N_BATCH)
        out[n] += res.results[c]["outp"]
    out += bo[None, None, :]
    return out
